# revision 16
# baseline (speedup 1.0000x reference)
"""Causal multi-head attention block on 8 Trainium2 NeuronCores.

Reference computation (per batch b):
    qkv = x @ w_attn + b_attn; split q,k,v; per head: S = q k^T / sqrt(hs),
    causal softmax, y = P v; out = concat(y) @ w_proj + b_proj.

Sharding: data parallel over batch. B == 8 == n_cores, so each core runs one
batch independently (no collectives). Each core gets the full weights and its
x[b] slice; outputs are stacked on the host.

Per-core dataflow (all matmuls in float32r: full PE rate, ~1e-4 rounding):
  xT = transpose(x) via PE                              [C, T]
  qkT = w_attn[:, :2C]^T-layout matmul:  lhsT=w tiles, rhs=xT  -> [2C, T]
        (n on partitions => per-head qT_h/kT_h are [64, T] slices)
  v   = lhsT=xT tiles, rhs=w_attn[:, 2C:] -> natural [T, C] (+ ones column
        for the softmax denominator trick)
  per head h, per i-chunk c (256 wide):
      ST block (j_tile, c) = lhsT=kT_h[j_tile] rhs=qT_h[chunk]  [128j, 256i]
      P^T = exp(0.125 * ST)   (no max subtraction: |S/8| <= ~7 bounded)
      causal: skip blocks fully above diagonal, 0/1-mask diagonal blocks
      yT_aug[65, 256] += v_aug_h[j_tile]^T @ P^T   (row 64 = row sums l)
      rl = 1/l; PE-broadcast rl to 64 partitions; yT = yT_u * rl
  out = lhsT=yT tiles, rhs=w_proj + b_proj -> [T, C]
"""

import numpy as np
from contextlib import ExitStack

import concourse.bass as bass
import concourse.mybir as mybir
import concourse.tile as tile
from concourse import bacc
from concourse.bass_utils import run_bass_kernel_spmd
from concourse.masks import make_identity

F32 = mybir.dt.float32
F32R = mybir.dt.float32r
AF = mybir.ActivationFunctionType

B, T, C = 8, 1024, 768
H, HS = 12, 64
KT = C // 128            # 6 contraction tiles
MT = T // 128            # 8 row tiles (also j tiles)
ICH = 256                # attention i-chunk width
NCH = T // ICH           # 4 chunks
SCALE = 1.0 / np.sqrt(HS)

N_CORES = 8


def build_program():
    nc = bacc.Bacc("TRN2", target_bir_lowering=False, debug=False)

    x = nc.dram_tensor("x", [T, C], F32, kind="ExternalInput")
    w_attn = nc.dram_tensor("w_attn", [C, 3 * C], F32, kind="ExternalInput")
    b_attn = nc.dram_tensor("b_attn", [3 * C], F32, kind="ExternalInput")
    w_proj = nc.dram_tensor("w_proj", [C, C], F32, kind="ExternalInput")
    b_proj = nc.dram_tensor("b_proj", [C], F32, kind="ExternalInput")
    out = nc.dram_tensor("out", [T, C], F32, kind="ExternalOutput")

    with tile.TileContext(nc) as tc, ExitStack() as ctx:
        consts = ctx.enter_context(tc.tile_pool(name="consts", bufs=1))
        big = ctx.enter_context(tc.tile_pool(name="big", bufs=1))
        io = ctx.enter_context(tc.tile_pool(name="io", bufs=3))
        wstage = ctx.enter_context(tc.tile_pool(name="wstage", bufs=2))
        pt_pool = ctx.enter_context(tc.tile_pool(name="pt", bufs=10))
        rl_pool = ctx.enter_context(tc.tile_pool(name="rl", bufs=3))
        ps_small = ctx.enter_context(tc.tile_pool(name="ps_small", bufs=2, space="PSUM"))
        ps_big = ctx.enter_context(tc.tile_pool(name="ps_big", bufs=2, space="PSUM"))
        ps_y = ctx.enter_context(tc.tile_pool(name="ps_y", bufs=4, space="PSUM"))

        # ---- constants ----
        ident = consts.tile([128, 128], F32, tag="ident")
        make_identity(nc, ident)
        # triangular 0/1 mask for diagonal blocks: keep p <= f
        tri = consts.tile([128, 128], F32, tag="tri")
        nc.gpsimd.memset(tri, 1.0)
        nc.gpsimd.affine_select(
            out=tri, in_=tri, compare_op=mybir.AluOpType.is_ge,
            fill=0.0, base=0, pattern=[[1, 128]], channel_multiplier=-1,
        )  # keep 1.0 where f - p >= 0 (upper triangle incl diag), 0 below
        # transposed per-partition bias view: battn_t[p, t] = b_attn[t*128+p]
        battn_t = consts.tile([128, 18], F32, tag="battn_t")
        nc.sync.dma_start(out=battn_t, in_=b_attn[:].rearrange("(t p) -> p t", p=128))
        # broadcast biases over partitions (DRAM source, stride-0 partition dim)
        bv_b = consts.tile([128, C], F32, tag="bias_b")
        nc.sync.dma_start(
            out=bv_b,
            in_=bass.AP(tensor=b_attn[:].tensor, offset=2 * C, ap=[[0, 128], [1, C]]),
        )

        # ---- weights: DMA k-tile into f32 staging, round into f32r tile ----
        # (the f32r verifier requires a compute-op producer for matmul inputs,
        # so DMA-then-in-place-bitcast is not allowed)
        WCH = 1152
        wr = big.tile([128, KT, 3 * C], F32R, tag="w_sb")
        w_dram = w_attn[:].rearrange("(t p) n -> p t n", p=128)
        for kt in range(KT):
            for c0 in range(0, 3 * C, WCH):
                stg = wstage.tile([128, WCH], F32, tag="wstage")
                nc.sync.dma_start(out=stg, in_=w_dram[:, kt, c0:c0 + WCH])
                nc.gpsimd.tensor_copy(wr[:, kt, c0:c0 + WCH], stg)

        wpr = big.tile([128, KT, C], F32R, tag="wp_sb")
        wp_dram = w_proj[:].rearrange("(t p) n -> p t n", p=128)
        for kt in range(KT):
            stg = wstage.tile([128, WCH], F32, tag="wstage")
            nc.sync.dma_start(out=stg[:, 0:C], in_=wp_dram[:, kt, :])
            nc.gpsimd.tensor_copy(wpr[:, kt, :], stg[:, 0:C])

        # ---- phase A: x load + transpose (streamed per m-tile) ----
        xT = big.tile([128, KT, T], F32R, tag="xT")
        for mt in range(MT):
            x_sb = io.tile([128, C], F32, tag="io")
            nc.sync.dma_start(out=x_sb, in_=x[mt * 128:(mt + 1) * 128, :])
            for kt in range(KT):
                pt = ps_small.tile([128, 256], F32, tag="st")
                nc.tensor.transpose(
                    pt[:, 0:128], x_sb[:, kt * 128:(kt + 1) * 128], ident)
                nc.vector.tensor_copy(xT[:, kt, mt * 128:(mt + 1) * 128], pt[:, 0:128])

        # ---- phase B: qkT [2C, T] and v_aug [T, H, 65] ----
        qk = big.tile([128, 12, T], F32R, tag="qk")
        for nt in range(12):
            for mc in range(2):
                ps = ps_big.tile([128, 512], F32, tag="mm")
                for kt in range(KT):
                    nc.tensor.matmul(
                        ps, wr[:, kt, nt * 128:(nt + 1) * 128],
                        xT[:, kt, mc * 512:(mc + 1) * 512],
                        start=(kt == 0), stop=(kt == KT - 1),
                    )
                nc.scalar.activation(
                    qk[:, nt, mc * 512:(mc + 1) * 512], ps, AF.Identity,
                    bias=battn_t[:, nt:nt + 1], scale=1.0,
                )

        v_aug = big.tile([128, MT, H, HS + 1], F32R, tag="v_aug")
        ones12 = consts.tile([128, H], F32, tag="ones12")
        nc.gpsimd.memset(ones12, 1.0)
        for mt in range(MT):
            nc.vector.tensor_copy(
                v_aug[:, mt, :, HS:HS + 1].rearrange("p h o -> p (h o)"), ones12)
            for nch, (n0, nsz) in enumerate([(0, 512), (512, 256)]):
                ps = ps_big.tile([128, 512], F32, tag="mm")
                for kt in range(KT):
                    nc.tensor.matmul(
                        ps[:, 0:nsz], xT[:, kt, mt * 128:(mt + 1) * 128],
                        wr[:, kt, 2 * C + n0:2 * C + n0 + nsz],
                        start=(kt == 0), stop=(kt == KT - 1),
                    )
                h0, nh = n0 // HS, nsz // HS
                nc.vector.tensor_add(
                    v_aug[:, mt, h0:h0 + nh, 0:HS],
                    ps[:, 0:nsz].rearrange("p (h d) -> p h d", d=HS),
                    bv_b[:, n0:n0 + nsz].rearrange("p (h d) -> p h d", d=HS),
                )

        # ---- phase C: attention per head ----
        # yT reuses xT's slot (same tag, bufs=1): xT is dead once phase B ends
        yT = big.tile([128, KT, T], F32R, tag="xT")
        for h in range(H):
            nt_q, po = h // 2, 64 * (h % 2)
            nt_k = 6 + h // 2
            qT_h = qk[po:po + 64, nt_q, :]
            kT_h = qk[po:po + 64, nt_k, :]
            for c in range(NCH):
                isl = slice(c * ICH, (c + 1) * ICH)
                n_jt = 2 * (c + 1)
                pts = []
                for jt in range(n_jt):
                    st = ps_small.tile([128, ICH], F32, tag="st")
                    nc.tensor.matmul(
                        st, kT_h[:, jt * 128:(jt + 1) * 128], qT_h[:, isl],
                        start=True, stop=True,
                    )
                    ptile = pt_pool.tile([128, ICH], F32R, tag="ptile")
                    diag = jt // 2 == c  # jt in {2c, 2c+1}
                    if diag and jt % 2 == 1:
                        # only columns 128: are (partially) valid; zero the rest
                        # via a DVE op (f32r consumers need compute producers)
                        nc.vector.tensor_scalar_mul(ptile[:, 0:128], tri, 0.0)
                        nc.scalar.activation(
                            ptile[:, 128:ICH], st[:, 128:ICH], AF.Exp,
                            bias=0.0, scale=SCALE)
                        nc.vector.tensor_mul(
                            ptile[:, 128:ICH], ptile[:, 128:ICH], tri)
                    else:
                        nc.scalar.activation(
                            ptile, st, AF.Exp, bias=0.0, scale=SCALE)
                        if diag:
                            nc.vector.tensor_mul(
                                ptile[:, 0:128], ptile[:, 0:128], tri)
                    pts.append(ptile)
                ya = ps_y.tile([HS + 1, ICH], F32, tag="ya")
                for jt in range(n_jt):
                    nc.tensor.matmul(
                        ya, v_aug[:, jt, h, :], pts[jt],
                        start=(jt == 0), stop=(jt == n_jt - 1),
                    )
                # rl = 1/l on partition 0, gpsimd-broadcast to 64 partitions
                rl = rl_pool.tile([1, ICH], F32, tag="rl")
                nc.vector.reciprocal(rl, ya[HS:HS + 1, :])
                rlb = rl_pool.tile([64, ICH], F32, tag="rlb")
                nc.gpsimd.partition_broadcast(rlb, rl)
                nc.vector.tensor_mul(
                    yT[po:po + 64, nt_q, isl], ya[0:HS, :], rlb)

        # ---- phase D: output projection (streamed per m-tile) ----
        bp_b = consts.tile([128, C], F32, tag="bias_b")  # reuses bv_b's slot
        nc.sync.dma_start(
            out=bp_b,
            in_=bass.AP(tensor=b_proj[:].tensor, offset=0, ap=[[0, 128], [1, C]]),
        )
        for mt in range(MT):
            out_sb = io.tile([128, C], F32, tag="io")
            for (c0, csz) in [(0, 512), (512, 256)]:
                ps = ps_big.tile([128, 512], F32, tag="mm")
                for nt in range(KT):
                    nc.tensor.matmul(
                        ps[:, 0:csz], yT[:, nt, mt * 128:(mt + 1) * 128],
                        wpr[:, nt, c0:c0 + csz],
                        start=(nt == 0), stop=(nt == KT - 1),
                    )
                nc.vector.tensor_add(
                    out_sb[:, c0:c0 + csz], ps[:, 0:csz], bp_b[:, c0:c0 + csz])
            nc.sync.dma_start(
                out=out[mt * 128:(mt + 1) * 128, :], in_=out_sb)

    nc.compile()
    return nc


_CACHE = {}


def _get_program():
    if "nc" not in _CACHE:
        _CACHE["nc"] = build_program()
    return _CACHE["nc"]


def kernel(x, w_attn, b_attn, w_proj, b_proj):
    nc = _get_program()
    x = np.asarray(x, dtype=np.float32)
    in_maps = [
        {
            "x": np.ascontiguousarray(x[b]),
            "w_attn": np.asarray(w_attn, np.float32),
            "b_attn": np.asarray(b_attn, np.float32),
            "w_proj": np.asarray(w_proj, np.float32),
            "b_proj": np.asarray(b_proj, np.float32),
        }
        for b in range(B)
    ]
    res = run_bass_kernel_spmd(nc, in_maps, list(range(N_CORES)))
    return np.stack([res.results[b]["out"] for b in range(B)], axis=0)
